# revision 7
# baseline (speedup 1.0000x reference)
"""3-layer GraphConv GNN encoder on 8 TRN2 NeuronCores.

Math (per reference):
    deg_out/deg_in from src/dst (clamped >=1), norm = deg^-0.5
    layer(x) = (D_dst^-1/2 A D_src^-1/2 x) W + b, relu after layers 0,1
    out = mean over nodes of layer3 output  -> [1, 128]

Distribution: dst-node sharding across 8 cores (12544 nodes/core).
Per layer each core gathers src-rows for its ~200k edges via dma_gather
(int16 window-relative indices over 4 windows of 25088 rows), reduces
edge chunks of 128 into per-128-dst-node-block PSUM tiles with a
TensorE matmul against a VectorE-built one-hot selector (edge weight
norm_src*norm_dst folded into the selector values), then applies the
dense W via a second matmul.  Layer-1 output (fp16) is AllGathered so
layer 2 can gather any row.  Layer 3 collapses to a host-precomputed
per-node weight c[s] = norm_src[s] * sum_{(s,d) in E} norm_dst[d]:
mean(L3) = (1/N) * (sum_s c[s] x2[s]) @ W2 + b2, so layer 2 fuses a
weighted column-sum (pool) and no second exchange is needed.  The 8
per-core partial pools are summed on the host.
"""

import numpy as np

N_REAL = 100000
D = 128
NCORES = 8
NS = 12544            # dst nodes per core
N_PAD = NS * NCORES   # 100352
NB = NS // 128        # 98 dst blocks per core
NWIN = 4
WINR = N_PAD // NWIN  # 25088 rows per index window (< 2^15)
SEG = 8               # dst blocks per gather call


def _preprocess(src, dst):
    """Pure graph-structure work (degrees, norms, edge buckets, index arrays)."""
    src = np.asarray(src).astype(np.int64)
    dst = np.asarray(dst).astype(np.int64)
    ones = np.ones(len(src), np.float32)
    deg_out = np.maximum(np.bincount(src, weights=ones, minlength=N_REAL), 1.0)
    deg_in = np.maximum(np.bincount(dst, weights=ones, minlength=N_REAL), 1.0)
    norm_src = (deg_out ** -0.5).astype(np.float32)
    norm_dst = (deg_in ** -0.5).astype(np.float32)
    w_edge = norm_src[src] * norm_dst[dst]

    # layer-3 pool weights (no 1/N; applied on host at the end)
    c_full = np.zeros(N_PAD, np.float32)
    c_full[:N_REAL] = norm_src * np.bincount(
        src, weights=norm_dst[dst], minlength=N_REAL
    ).astype(np.float32)

    core = dst // NS
    block = (dst % NS) // 128
    win = src // WINR
    dstrel = (dst % NS) % 128

    # bucket edges by (core, block, window); sort each bucket by src
    key = ((core * NB + block) * NWIN + win).astype(np.int64)
    order = np.lexsort((src, key))
    key_s = key[order]
    counts = np.bincount(key_s, minlength=NCORES * NB * NWIN)
    nch = int(np.ceil(counts.max() / 128.0))
    cap = nch * 128

    # scatter edges into fixed-capacity slots: flat layout per core is
    # [w, b, cap] so a gather call (w, blocks b0..b0+S) is contiguous
    starts = np.zeros(NCORES * NB * NWIN + 1, np.int64)
    np.cumsum(counts, out=starts[1:])
    within = np.arange(len(src)) - starts[key_s]

    co = key_s // (NB * NWIN)
    bl = (key_s // NWIN) % NB
    wi = key_s % NWIN
    slot = ((co * NWIN + wi) * NB + bl) * cap + within  # global slot id

    tot = NWIN * NB * cap  # slots per core
    idx_flat = np.zeros(NCORES * tot, np.int16)      # window-relative src, pad 0
    drel_flat = np.zeros(NCORES * tot, np.float32)   # dst rel to block, pad 0
    w_flat = np.zeros(NCORES * tot, np.float32)      # edge weight, pad 0
    idx_flat[slot] = (src[order] % WINR).astype(np.int16)
    drel_flat[slot] = dstrel[order].astype(np.float32)
    w_flat[slot] = w_edge[order]

    per_core = []
    for cidx in range(NCORES):
        fl = idx_flat[cidx * tot:(cidx + 1) * tot]
        # wrap for dma_gather: index i -> [i % 16, i // 16], replicated x8
        wrapped = np.tile(np.ascontiguousarray(fl.reshape(-1, 16).T), (8, 1))
        dr = drel_flat[cidx * tot:(cidx + 1) * tot].reshape(-1, 128).T  # [128, chunks]
        wv = w_flat[cidx * tot:(cidx + 1) * tot].reshape(-1, 128).T
        cv = c_full[cidx * NS:(cidx + 1) * NS].reshape(NB, 128).T       # [128, NB]
        per_core.append({
            "idx": np.ascontiguousarray(wrapped),
            "drel": np.ascontiguousarray(dr),
            "wv": np.ascontiguousarray(wv),
            "cv": np.ascontiguousarray(cv.astype(np.float16)),
        })
    return nch, per_core


def _build_program(nch):
    from concourse import bass, bacc, mybir
    import concourse.tile as tile

    f32, f16, i16 = mybir.dt.float32, mybir.dt.float16, mybir.dt.int16
    cap = nch * 128
    ncols = NB * NWIN * nch          # selector columns (chunks) per layer
    nidx16 = NWIN * NB * cap // 16   # wrapped index columns

    nc = bacc.Bacc("TRN2", target_bir_lowering=False, debug=False,
                   num_devices=NCORES)

    h_in = nc.dram_tensor("h", [N_PAD, D], f32, kind="ExternalInput")
    idx_in = nc.dram_tensor("idx", [128, nidx16], i16, kind="ExternalInput")
    drel_in = nc.dram_tensor("drel", [128, ncols], f32, kind="ExternalInput")
    wv_in = nc.dram_tensor("wv", [128, ncols], f32, kind="ExternalInput")
    cv_in = nc.dram_tensor("cv", [128, NB], f16, kind="ExternalInput")
    iota_in = nc.dram_tensor("iota", [128, 128], f16, kind="ExternalInput")
    w0_in = nc.dram_tensor("W0", [D, D], f32, kind="ExternalInput")
    w1_in = nc.dram_tensor("W1", [D, D], f32, kind="ExternalInput")
    w2_in = nc.dram_tensor("W2", [D, D], f32, kind="ExternalInput")
    out_ext = nc.dram_tensor("out", [1, D], f32, kind="ExternalOutput")

    h16 = nc.dram_tensor("h16", [N_PAD, D], f16)
    x1_shard = nc.dram_tensor("x1_shard", [NS, D], f16)
    x1_full = nc.dram_tensor("x1_full", [N_PAD, D], f16, addr_space="Shared")

    # gather calls: 8 chunks (1024 indices) per call, per window
    CPG = 8  # chunks per gather call (1024 idx = HW dma_gather limit)
    wchunks = NB * nch                      # chunks per window
    calls_per_win = -(-wchunks // CPG)

    with tile.TileContext(nc) as tc:
        with (
            tc.tile_pool(name="const", bufs=1) as constp,
            tc.tile_pool(name="wtile", bufs=16) as wtp,
            tc.tile_pool(name="sel", bufs=4) as selp,
            tc.tile_pool(name="epi", bufs=3) as epip,
            tc.tile_pool(name="psum1", bufs=2, space="PSUM") as ps1,
            tc.tile_pool(name="psum2", bufs=2, space="PSUM") as ps2,
            tc.tile_pool(name="psumS", bufs=1, space="PSUM") as psS,
        ):
            # resident inputs
            idx_sb = constp.tile([128, nidx16], i16)
            nc.sync.dma_start(out=idx_sb[:], in_=idx_in[:])
            drel_sb = constp.tile([128, ncols], f32)
            nc.sync.dma_start(out=drel_sb[:], in_=drel_in[:])
            wv_sb = constp.tile([128, ncols], f32)
            nc.sync.dma_start(out=wv_sb[:], in_=wv_in[:])
            cv_sb = constp.tile([128, NB], f16)
            nc.sync.dma_start(out=cv_sb[:], in_=cv_in[:])
            iota_sb = constp.tile([128, 128], f16)
            nc.sync.dma_start(out=iota_sb[:], in_=iota_in[:])

            wmats = []
            for w_in in (w0_in, w1_in, w2_in):
                wf = epip.tile([D, D], f32, tag="wload")
                nc.sync.dma_start(out=wf[:], in_=w_in[:])
                w16 = constp.tile([D, D], f16)
                nc.vector.tensor_copy(out=w16[:], in_=wf[:])
                wmats.append(w16)

            # h -> fp16 (cast during SWDGE DMA), one call per window
            for w in range(NWIN):
                nc.gpsimd.dma_start(
                    out=h16[w * WINR:(w + 1) * WINR, :],
                    in_=h_in[w * WINR:(w + 1) * WINR, :],
                )

            pool_ps = psS.tile([128, 1], mybir.dt.float32, space="PSUM")

            def layer(src_dram, lidx):
                """lidx 0: write x1_shard; lidx 1: fused pool accumulation."""
                wmat = wmats[lidx]
                tiles = [[None] * calls_per_win for _ in range(NWIN)]
                emitted = [0] * NWIN

                def ensure_calls(w, up_to):
                    while emitted[w] <= up_to:
                        call = emitted[w]
                        c0 = call * CPG
                        ncg = min(CPG, wchunks - c0)
                        gt = wtp.tile([128, CPG, D], f16, tag="wt")
                        icol0 = (w * wchunks + c0) * 128 // 16
                        nc.gpsimd.dma_gather(
                            out_ap=gt[:, :ncg, :],
                            in_ap=src_dram[w * WINR:(w + 1) * WINR, :],
                            idxs_ap=idx_sb[:, icol0:icol0 + ncg * 8],
                            num_idxs=ncg * 128, num_idxs_reg=ncg * 128,
                            elem_size=D, queue_num=0,
                        )
                        tiles[w][call] = gt
                        emitted[w] += 1

                for b in range(NB):
                    last_call = ((b + 1) * nch - 1) // CPG
                    for w in range(NWIN):
                        ensure_calls(w, last_call)
                    p1 = ps1.tile([128, 128], mybir.dt.float32, space="PSUM")
                    first = True
                    prev_mm = None
                    for w in range(NWIN):
                        for cch in range(nch):
                            g = b * nch + cch
                            gt = tiles[w][g // CPG]
                            col = (w * NB + b) * nch + cch
                            s_t = selp.tile([128, 128], f16, tag="sel")
                            nc.vector.tensor_scalar(
                                out=s_t[:], in0=iota_sb[:],
                                scalar1=drel_sb[:, col:col + 1],
                                scalar2=wv_sb[:, col:col + 1],
                                op0=mybir.AluOpType.is_equal,
                                op1=mybir.AluOpType.mult,
                            )
                            mm = nc.tensor.matmul(
                                out=p1[:],
                                lhsT=gt[:, g % CPG, :],
                                rhs=s_t[:],
                                start=first,
                                stop=(w == NWIN - 1 and cch == nch - 1),
                            )
                            if prev_mm is not None:
                                # keep PSUM accumulation-group order on PE
                                tile.add_dep_helper(mm.ins, prev_mm.ins,
                                                    sync=False,
                                                    reason="psum accum order")
                            prev_mm = mm
                            first = False
                    # p1 = aggT [fin x dst]; cast to fp16
                    aggT = epip.tile([128, 128], f16, tag="aggT")
                    nc.vector.tensor_copy(out=aggT[:], in_=p1[:])
                    p2 = ps2.tile([128, 128], mybir.dt.float32, space="PSUM")
                    nc.tensor.matmul(out=p2[:], lhsT=aggT[:], rhs=wmat[:],
                                     start=True, stop=True)
                    xt = epip.tile([128, 128], f16, tag="xt")
                    nc.vector.tensor_scalar(
                        out=xt[:], in0=p2[:], scalar1=0.0, scalar2=None,
                        op0=mybir.AluOpType.max,
                    )
                    if lidx == 0:
                        nc.sync.dma_start(
                            out=x1_shard[b * 128:(b + 1) * 128, :], in_=xt[:])
                    else:
                        nc.tensor.matmul(
                            out=pool_ps[:], lhsT=xt[:],
                            rhs=cv_sb[:, b:b + 1],
                            start=(b == 0), stop=(b == NB - 1),
                        )

            layer(h16, 0)
            nc.gpsimd.collective_compute(
                "AllGather", mybir.AluOpType.bypass,
                replica_groups=[list(range(NCORES))],
                ins=[x1_shard[:]], outs=[x1_full[:]],
            )
            layer(x1_full, 1)

            # finalize: out = (pool^T @ W2) as [1, 128]
            poolv = constp.tile([128, 1], f16)
            nc.vector.tensor_copy(out=poolv[:], in_=pool_ps[:])
            pout = psS.tile([1, 128], mybir.dt.float32, space="PSUM")
            nc.tensor.matmul(out=pout[:], lhsT=poolv[:], rhs=wmats[2][:],
                             start=True, stop=True)
            ovec = constp.tile([1, 128], f32)
            nc.vector.tensor_copy(out=ovec[:], in_=pout[:])
            nc.sync.dma_start(out=out_ext[:], in_=ovec[:])

    nc.compile()
    return nc


_CACHE = {}


def _get_program(nch):
    if nch not in _CACHE:
        _CACHE[nch] = _build_program(nch)
    return _CACHE[nch]


def kernel(h, src, dst, W0, b0, W1, b1, W2, b2):
    from concourse.bass_utils import run_bass_kernel_spmd

    h = np.asarray(h, np.float32)
    W0 = np.asarray(W0, np.float32)
    W1 = np.asarray(W1, np.float32)
    W2 = np.asarray(W2, np.float32)
    b0 = np.asarray(b0, np.float32)
    b1 = np.asarray(b1, np.float32)
    b2 = np.asarray(b2, np.float32)
    assert not (np.any(b0) or np.any(b1)), "nonzero b0/b1 not wired in"

    nch, per_core = _preprocess(src, dst)
    nc = _get_program(nch)

    h_pad = np.zeros((N_PAD, D), np.float32)
    h_pad[:N_REAL] = h
    iota = np.tile(np.arange(128, dtype=np.float16), (128, 1))

    in_maps = []
    for c in range(NCORES):
        pc = per_core[c]
        in_maps.append({
            "h": h_pad, "idx": pc["idx"], "drel": pc["drel"], "wv": pc["wv"],
            "cv": pc["cv"], "iota": iota, "W0": W0, "W1": W1, "W2": W2,
        })
    res = run_bass_kernel_spmd(nc, in_maps, list(range(NCORES)))
    total = np.zeros((1, D), np.float64)
    for c in range(NCORES):
        total += res.results[c]["out"].astype(np.float64)
    out = (total / float(N_REAL)) + b2.astype(np.float64)
    return out.astype(np.float32)


# revision 11
# speedup vs baseline: 24.9020x; 24.9020x over previous
"""3-layer GraphConv GNN encoder on 8 TRN2 NeuronCores.

Math (per reference):
    deg_out/deg_in from src/dst (clamped >=1), norm = deg^-0.5
    layer(x) = (D_dst^-1/2 A D_src^-1/2 x) W + b, relu after layers 0,1
    out = mean over nodes of layer3 output  -> [1, 128]

Distribution: dst-node sharding across 8 cores (12544 nodes/core).
Per layer each core gathers src-rows for its ~200k edges via dma_gather
(int16 window-relative indices over 4 windows of 25088 rows), reduces
edge chunks of 128 into per-128-dst-node-block PSUM tiles with a
TensorE matmul against a VectorE-built one-hot selector (edge weight
norm_src*norm_dst folded into the selector values), then applies the
dense W via a second matmul.  Layer-1 output (fp16) is AllGathered so
layer 2 can gather any row.  Layer 3 collapses to a host-precomputed
per-node weight c[s] = norm_src[s] * sum_{(s,d) in E} norm_dst[d]:
mean(L3) = (1/N) * (sum_s c[s] x2[s]) @ W2 + b2, so layer 2 fuses a
weighted column-sum (pool) and no second exchange is needed.  The 8
per-core partial pools are summed on the host.
"""

import numpy as np

N_REAL = 100000
D = 128
NCORES = 8
NS = 12544            # dst nodes per core
N_PAD = NS * NCORES   # 100352
NB = NS // 128        # 98 dst blocks per core
NWIN = 4
WINR = N_PAD // NWIN  # 25088 rows per index window (< 2^15)
SEG = 8               # dst blocks per gather call


def _preprocess(src, dst):
    """Pure graph-structure work (degrees, norms, edge buckets, index arrays)."""
    src = np.asarray(src).astype(np.int64)
    dst = np.asarray(dst).astype(np.int64)
    ones = np.ones(len(src), np.float32)
    deg_out = np.maximum(np.bincount(src, weights=ones, minlength=N_REAL), 1.0)
    deg_in = np.maximum(np.bincount(dst, weights=ones, minlength=N_REAL), 1.0)
    norm_src = (deg_out ** -0.5).astype(np.float32)
    norm_dst = (deg_in ** -0.5).astype(np.float32)
    w_edge = norm_src[src] * norm_dst[dst]

    # layer-3 pool weights (no 1/N; applied on host at the end)
    c_full = np.zeros(N_PAD, np.float32)
    c_full[:N_REAL] = norm_src * np.bincount(
        src, weights=norm_dst[dst], minlength=N_REAL
    ).astype(np.float32)

    core = dst // NS
    block = (dst % NS) // 128
    win = src // WINR
    dstrel = (dst % NS) % 128

    # bucket edges by (core, block, window); sort each bucket by src
    key = ((core * NB + block) * NWIN + win).astype(np.int64)
    order = np.lexsort((src, key))
    key_s = key[order]
    counts = np.bincount(key_s, minlength=NCORES * NB * NWIN)
    nch = int(np.ceil(counts.max() / 128.0))
    cap = nch * 128

    # scatter edges into fixed-capacity slots: flat layout per core is
    # [w, b, cap] so a gather call (w, blocks b0..b0+S) is contiguous
    starts = np.zeros(NCORES * NB * NWIN + 1, np.int64)
    np.cumsum(counts, out=starts[1:])
    within = np.arange(len(src)) - starts[key_s]

    co = key_s // (NB * NWIN)
    bl = (key_s // NWIN) % NB
    wi = key_s % NWIN
    slot = ((co * NWIN + wi) * NB + bl) * cap + within  # global slot id

    tot = NWIN * NB * cap  # slots per core
    idx_flat = np.zeros(NCORES * tot, np.int16)      # window-relative src, pad 0
    drel_flat = np.zeros(NCORES * tot, np.float32)   # dst rel to block, pad 0
    w_flat = np.zeros(NCORES * tot, np.float32)      # edge weight, pad 0
    idx_flat[slot] = (src[order] % WINR).astype(np.int16)
    drel_flat[slot] = dstrel[order].astype(np.float32)
    w_flat[slot] = w_edge[order]

    per_core = []
    for cidx in range(NCORES):
        fl = idx_flat[cidx * tot:(cidx + 1) * tot]
        # wrap for dma_gather: index i -> [i % 16, i // 16], replicated x8
        wrapped = np.tile(np.ascontiguousarray(fl.reshape(-1, 16).T), (8, 1))
        dr = drel_flat[cidx * tot:(cidx + 1) * tot].reshape(-1, 128).T  # [128, chunks]
        wv = w_flat[cidx * tot:(cidx + 1) * tot].reshape(-1, 128).T
        cv = c_full[cidx * NS:(cidx + 1) * NS].reshape(NB, 128).T       # [128, NB]
        per_core.append({
            "idx": np.ascontiguousarray(wrapped),
            "drel": np.ascontiguousarray(dr),
            "wv": np.ascontiguousarray(wv),
            "cv": np.ascontiguousarray(cv.astype(np.float16)),
        })
    return nch, per_core


def _build_program(nch, repeat=1):
    from concourse import bass, bacc, mybir
    import concourse.tile as tile

    f32, f16, i16 = mybir.dt.float32, mybir.dt.float16, mybir.dt.int16
    cap = nch * 128
    ncols = NB * NWIN * nch          # selector columns (chunks) per layer
    nidx16 = NWIN * NB * cap // 16   # wrapped index columns

    nc = bacc.Bacc("TRN2", target_bir_lowering=False, debug=False,
                   num_devices=NCORES)

    h_in = nc.dram_tensor("h", [N_PAD, D], f32, kind="ExternalInput")
    idx_in = nc.dram_tensor("idx", [128, nidx16], i16, kind="ExternalInput")
    drel_in = nc.dram_tensor("drel", [128, ncols], f32, kind="ExternalInput")
    wv_in = nc.dram_tensor("wv", [128, ncols], f32, kind="ExternalInput")
    cv_in = nc.dram_tensor("cv", [128, NB], f16, kind="ExternalInput")
    iota_in = nc.dram_tensor("iota", [128, 128], f16, kind="ExternalInput")
    w0_in = nc.dram_tensor("W0", [D, D], f32, kind="ExternalInput")
    w1_in = nc.dram_tensor("W1", [D, D], f32, kind="ExternalInput")
    w2_in = nc.dram_tensor("W2", [D, D], f32, kind="ExternalInput")
    out_ext = nc.dram_tensor("out", [1, D], f32, kind="ExternalOutput")

    h16 = nc.dram_tensor("h16", [N_PAD, D], f16)
    x1_shard = nc.dram_tensor("x1_shard", [NS, D], f16)
    x1_full = nc.dram_tensor("x1_full", [N_PAD, D], f16, addr_space="Shared")

    # gather calls: 8 chunks (1024 indices) per call, per window
    CPG = 8  # chunks per gather call (1024 idx = HW dma_gather limit)
    wchunks = NB * nch                      # chunks per window
    calls_per_win = -(-wchunks // CPG)

    with tile.TileContext(nc) as tc:
        with (
            tc.tile_pool(name="const", bufs=1) as constp,
            tc.tile_pool(name="wtile", bufs=16) as wtp,
            tc.tile_pool(name="sel", bufs=4) as selp,
            tc.tile_pool(name="epi", bufs=3) as epip,
            tc.tile_pool(name="psum1", bufs=2, space="PSUM") as ps1,
            tc.tile_pool(name="psum2", bufs=2, space="PSUM") as ps2,
            tc.tile_pool(name="psumS", bufs=1, space="PSUM") as psS,
        ):
            # resident inputs
            idx_sb = constp.tile([128, nidx16], i16)
            nc.sync.dma_start(out=idx_sb[:], in_=idx_in[:])
            drel_sb = constp.tile([128, ncols], f32)
            nc.sync.dma_start(out=drel_sb[:], in_=drel_in[:])
            wv_sb = constp.tile([128, ncols], f32)
            nc.sync.dma_start(out=wv_sb[:], in_=wv_in[:])
            cv_sb = constp.tile([128, NB], f16)
            nc.sync.dma_start(out=cv_sb[:], in_=cv_in[:])
            iota_sb = constp.tile([128, 128], f16)
            nc.sync.dma_start(out=iota_sb[:], in_=iota_in[:])

            wmats = []
            for wi, w_in in enumerate((w0_in, w1_in, w2_in)):
                wf = epip.tile([D, D], f32, tag="wload")
                nc.sync.dma_start(out=wf[:], in_=w_in[:])
                w16 = constp.tile([D, D], f16, tag=f"w16_{wi}")
                nc.vector.tensor_copy(out=w16[:], in_=wf[:])
                wmats.append(w16)

            def layer(src_dram, lidx, pool_ps):
                """lidx 0: write x1_shard; lidx 1: fused pool accumulation."""
                wmat = wmats[lidx]
                tiles = [[None] * calls_per_win for _ in range(NWIN)]
                emitted = [0] * NWIN

                def ensure_calls(w, up_to):
                    while emitted[w] <= up_to:
                        call = emitted[w]
                        c0 = call * CPG
                        ncg = min(CPG, wchunks - c0)
                        gt = wtp.tile([128, CPG, D], f16, tag="wt")
                        icol0 = (w * wchunks + c0) * 128 // 16
                        nc.gpsimd.dma_gather(
                            out_ap=gt[:, :ncg, :],
                            in_ap=src_dram[w * WINR:(w + 1) * WINR, :],
                            idxs_ap=idx_sb[:, icol0:icol0 + ncg * 8],
                            num_idxs=ncg * 128, num_idxs_reg=ncg * 128,
                            elem_size=D, queue_num=0,
                        )
                        tiles[w][call] = gt
                        emitted[w] += 1

                for b in range(NB):
                    last_call = ((b + 1) * nch - 1) // CPG
                    for w in range(NWIN):
                        ensure_calls(w, last_call)
                    p1 = ps1.tile([128, 128], mybir.dt.float32, space="PSUM")
                    first = True
                    prev_mm = None
                    for w in range(NWIN):
                        for cch in range(nch):
                            g = b * nch + cch
                            gt = tiles[w][g // CPG]
                            col = (w * NB + b) * nch + cch
                            s_t = selp.tile([128, 128], f16, tag="sel")
                            nc.vector.tensor_scalar(
                                out=s_t[:], in0=iota_sb[:],
                                scalar1=drel_sb[:, col:col + 1],
                                scalar2=wv_sb[:, col:col + 1],
                                op0=mybir.AluOpType.is_equal,
                                op1=mybir.AluOpType.mult,
                            )
                            mm = nc.tensor.matmul(
                                out=p1[:],
                                lhsT=gt[:, g % CPG, :],
                                rhs=s_t[:],
                                start=first,
                                stop=(w == NWIN - 1 and cch == nch - 1),
                            )
                            if prev_mm is not None:
                                # keep PSUM accumulation-group order on PE
                                tile.add_dep_helper(mm.ins, prev_mm.ins,
                                                    sync=False,
                                                    reason="psum accum order")
                            prev_mm = mm
                            first = False
                    # p1 = aggT [fin x dst]; cast to fp16
                    aggT = epip.tile([128, 128], f16, tag="aggT")
                    nc.vector.tensor_copy(out=aggT[:], in_=p1[:])
                    p2 = ps2.tile([128, 128], mybir.dt.float32, space="PSUM")
                    nc.tensor.matmul(out=p2[:], lhsT=aggT[:], rhs=wmat[:],
                                     start=True, stop=True)
                    xt = epip.tile([128, 128], f16, tag="xt")
                    nc.vector.tensor_scalar(
                        out=xt[:], in0=p2[:], scalar1=0.0, scalar2=None,
                        op0=mybir.AluOpType.max,
                    )
                    if lidx == 0:
                        nc.sync.dma_start(
                            out=x1_shard[b * 128:(b + 1) * 128, :], in_=xt[:])
                    else:
                        nc.tensor.matmul(
                            out=pool_ps[:], lhsT=xt[:],
                            rhs=cv_sb[:, b:b + 1],
                            start=(b == 0), stop=(b == NB - 1),
                        )

            for _rep in range(repeat):
                # h -> fp16 (cast during SWDGE DMA), one call per window
                for w in range(NWIN):
                    nc.gpsimd.dma_start(
                        out=h16[w * WINR:(w + 1) * WINR, :],
                        in_=h_in[w * WINR:(w + 1) * WINR, :],
                    )
                pool_ps = psS.tile([128, 1], mybir.dt.float32, space="PSUM",
                                   tag="pool_ps")
                layer(h16, 0, pool_ps)
                nc.gpsimd.collective_compute(
                    "AllGather", mybir.AluOpType.bypass,
                    replica_groups=[list(range(NCORES))],
                    ins=[x1_shard[:]], outs=[x1_full[:]],
                )
                layer(x1_full, 1, pool_ps)

                # finalize: out = (pool^T @ W2) as [1, 128]
                poolv = epip.tile([128, 1], f16, tag="poolv")
                nc.vector.tensor_copy(out=poolv[:], in_=pool_ps[:])
                pout = psS.tile([1, 128], mybir.dt.float32, space="PSUM",
                                tag="pout")
                nc.tensor.matmul(out=pout[:], lhsT=poolv[:], rhs=wmats[2][:],
                                 start=True, stop=True)
                ovec = epip.tile([1, 128], f32, tag="ovec")
                nc.vector.tensor_copy(out=ovec[:], in_=pout[:])
                nc.sync.dma_start(out=out_ext[:], in_=ovec[:])

        _, snap = tc.schedule_and_allocate()
        nc._tile_est_ns = snap.time if snap is not None else None

    print(f"[kernel] tile cost-model makespan estimate: "
          f"{getattr(nc, '_tile_est_ns', None)} ns", flush=True)
    nc.compile()
    return nc


_CACHE = {}


def _get_program(nch, repeat=1):
    key = (nch, repeat)
    if key not in _CACHE:
        _CACHE[key] = _build_program(nch, repeat)
    return _CACHE[key]


def kernel(h, src, dst, W0, b0, W1, b1, W2, b2):
    from concourse.bass_utils import run_bass_kernel_spmd

    h = np.asarray(h, np.float32)
    W0 = np.asarray(W0, np.float32)
    W1 = np.asarray(W1, np.float32)
    W2 = np.asarray(W2, np.float32)
    b0 = np.asarray(b0, np.float32)
    b1 = np.asarray(b1, np.float32)
    b2 = np.asarray(b2, np.float32)
    assert not (np.any(b0) or np.any(b1)), "nonzero b0/b1 not wired in"

    nch, per_core = _preprocess(src, dst)
    nc = _get_program(nch)

    h_pad = np.zeros((N_PAD, D), np.float32)
    h_pad[:N_REAL] = h
    iota = np.tile(np.arange(128, dtype=np.float16), (128, 1))

    in_maps = []
    for c in range(NCORES):
        pc = per_core[c]
        in_maps.append({
            "h": h_pad, "idx": pc["idx"], "drel": pc["drel"], "wv": pc["wv"],
            "cv": pc["cv"], "iota": iota, "W0": W0, "W1": W1, "W2": W2,
        })
    res = run_bass_kernel_spmd(nc, in_maps, list(range(NCORES)))
    total = np.zeros((1, D), np.float64)
    for c in range(NCORES):
        total += res.results[c]["out"].astype(np.float64)
    out = (total / float(N_REAL)) + b2.astype(np.float64)
    return out.astype(np.float32)
